# revision 15
# baseline (speedup 1.0000x reference)
"""GQA multi-head attention, tensor-parallel over 8 TRN2 NeuronCores.

Problem (hardcoded): B=2, S=2048, DM=2048, H=32 query heads, G=8 KV
groups, D=64 head dim, causal softmax attention + out projection.

Sharding: core d owns KV group d and query heads 4d..4d+3. Each core
computes its Q/K/V projections, causal attention for its 4 heads, and
its slice of the out projection (a [B*S, DM] partial). The host sums
the 8 partials (the all-reduce) and adds the output bias.

Per-core layout (all matmuls in fp32r, 1 cycle/row at N=512):
  - xT [DM, B*S] streamed from DRAM; projections computed per 512-col
    s-block with the dm-contraction (16 k-tiles) accumulated in PSUM.
  - QT [hd, s] chunks (2x [128, S] per batch), KVT [128, S] with rows
    0-63 = K^T and 64-127 = V^T (K and V weights packed column-wise so
    one matmul produces both).
  - K^T duplicated to partitions 64-127 (DMA) so score matmuls for head
    pairs run row-packed (two concurrent K=64 matmuls in the PE array).
  - scores^T [t, sq] per head; exp on ScalarE (scale=1/8 fused); causal
    mask applied only on diagonal tiles via a precomputed 0/1 mask
    (no max-subtraction: scores are O(1) by construction, exp is safe).
  - V^T transposed back to V [t, d] on the PE and augmented with a ones
    column, so P@V_aug also yields the softmax denominator (row 64).
  - normalize: reciprocal of denom row, DMA partition-broadcast,
    elementwise multiply.
  - out projection: 4 accumulating K=64 matmuls per output tile.
"""

import os
import sys

import numpy as np

sys.path.insert(0, "/opt/trn_rl_repo")

import concourse.bass as bass  # noqa: E402
import concourse.tile as tile  # noqa: E402
from concourse import bacc, mybir  # noqa: E402
from concourse.bass_utils import run_bass_kernel_spmd  # noqa: E402
from concourse.masks import make_identity  # noqa: E402

B, S, DM = 2, 2048, 2048
H, G, D = 32, 8, 64
NCORES = 8
HPC = H // G  # 4 query heads per core
KT = DM // 128  # 16 dm k-tiles
SB = 512  # s-block (matmul moving free dim)

F32 = mybir.dt.float32
F32R = mybir.dt.float32r
EXP = mybir.ActivationFunctionType.Exp


def _r(ap):
    return ap.bitcast(F32R)


def build_program(b_count=B, s_len=S):
    """Build the per-core Bass program (same program on all 8 cores)."""
    nc = bacc.Bacc(
        "TRN2",
        target_bir_lowering=False,
        debug=False,
        enable_asserts=False,
        num_devices=NCORES,
    )
    bs = b_count * s_len
    nsb = s_len // SB  # s-blocks per batch
    ntt_b = s_len // 128  # t-tiles per batch

    xt = nc.dram_tensor("xt", [DM, bs], F32R, kind="ExternalInput").ap()
    wq = nc.dram_tensor("wq", [128, KT, 2 * 128], F32R, kind="ExternalInput").ap()
    wkv = nc.dram_tensor("wkv", [128, KT, 128], F32R, kind="ExternalInput").ap()
    wo4 = nc.dram_tensor("wo4", [128, HPC, DM], F32R, kind="ExternalInput").ap()
    bq2 = nc.dram_tensor("bq2", [128, 2], F32, kind="ExternalInput").ap()
    bkv = nc.dram_tensor("bkv", [128, 1], F32, kind="ExternalInput").ap()
    msk = nc.dram_tensor("msk", [128, 896], F32, kind="ExternalInput").ap()
    y = nc.dram_tensor("y", [bs, DM], F32, kind="ExternalOutput").ap()

    with tile.TileContext(nc) as tc:
        _emit(tc, y, xt, wq, wkv, wo4, bq2, bkv, msk, b_count, s_len, nsb, ntt_b)
    nc.compile()
    return nc


def _emit(tc, y, xt, wq, wkv, wo4, bq2, bkv, msk, b_count, s_len, nsb, ntt_b):
    nc = tc.nc
    with (
        tc.tile_pool(name="res", bufs=1) as res,
        tc.tile_pool(name="perb", bufs=1) as perb,
        tc.tile_pool(name="stream", bufs=2) as stream,
        tc.tile_pool(name="pp", bufs=3, space="PSUM") as pp,
    ):
        wq_s = res.tile([128, KT, 256], F32R, name="wq_s")
        nc.sync.dma_start(out=wq_s, in_=wq)
        wkv_s = res.tile([128, KT, 128], F32R, name="wkv_s")
        nc.sync.dma_start(out=wkv_s, in_=wkv)
        wo4_s = res.tile([128, HPC, DM], F32R, name="wo4_s")
        nc.sync.dma_start(out=wo4_s, in_=wo4)
        msk_s = res.tile([128, 896], F32, name="msk_s")
        nc.sync.dma_start(out=msk_s, in_=msk)
        bq_s = res.tile([128, 2], F32, name="bq_s")
        nc.sync.dma_start(out=bq_s, in_=bq2)
        bkv_s = res.tile([128, 1], F32, name="bkv_s")
        nc.sync.dma_start(out=bkv_s, in_=bkv)
        ident = res.tile([128, 128], F32, name="ident")
        make_identity(nc, ident)
        ones97f = res.tile([97, 64], F32, name="ones97f")
        nc.vector.memset(ones97f, 1.0)
        ones97 = res.tile([97, 64], F32R, name="ones97")
        nc.vector.tensor_copy(ones97, ones97f)
        onec = res.tile([128, 1], F32, name="onec")
        nc.vector.memset(onec, 1.0)

        pending = []

        def drain(n):
            for _ in range(min(n, len(pending))):
                pending.pop(0)()

        for b in range(b_count):
            bs0 = b * s_len
            qc = [
                perb.tile([128, s_len], F32R, tag=f"qc{c}", name=f"qc{c}_{b}")
                for c in range(2)
            ]
            kvt = perb.tile([128, s_len], F32R, tag="kvt", name=f"kvt_{b}")
            kz = perb.tile([128, s_len], F32R, tag="kz", name=f"kz_{b}")
            kz2 = perb.tile([128, s_len], F32R, tag="kdup", name=f"kz2_{b}")
            vaug = []

            # ---- projections, per s-block ----
            for sb in range(nsb):
                s0 = sb * SB
                g0 = bs0 + s0
                xts = []
                for k in range(KT):
                    xtt = stream.tile(
                        [128, SB], F32R, tag="xt", bufs=18, name=f"xt_{b}_{sb}_{k}"
                    )
                    nc.sync.dma_start(out=xtt, in_=xt[k * 128 : (k + 1) * 128, g0 : g0 + SB])
                    xts.append(xtt)
                for c in range(2):
                    ps = pp.tile([128, SB], F32, tag="work", name=f"psq_{b}_{sb}_{c}")
                    for k in range(KT):
                        nc.tensor.matmul(
                            ps,
                            wq_s[:, k, c * 128 : (c + 1) * 128],
                            xts[k],
                            start=(k == 0),
                            stop=(k == KT - 1),
                        )
                    nc.vector.tensor_scalar_add(
                        qc[c][:, s0 : s0 + SB], ps, bq_s[:, c : c + 1]
                    )
                ps = pp.tile([128, SB], F32, tag="work", name=f"pskv_{b}_{sb}")
                for k in range(KT):
                    nc.tensor.matmul(
                        ps,
                        wkv_s[:, k, :],
                        xts[k],
                        start=(k == 0),
                        stop=(k == KT - 1),
                    )
                nc.vector.tensor_scalar_add(
                    kvt[:, s0 : s0 + SB], ps, bkv_s[:, 0:1]
                )
                # zero-padded K^T tiles: kz = [K^T; 0], kz2 = [0; K^T].
                # K=128 score matmuls keep the PE activity monitor warm
                # (K=64 never un-throttles the clock gate); junk Q rows meet
                # zero weights so results are exact.
                nc.sync.dma_start(
                    out=kz[0:64, s0 : s0 + SB], in_=kvt[0:64, s0 : s0 + SB]
                )
                nc.vector.tensor_scalar_mul(
                    kz[64:128, s0 : s0 + SB], kvt[64:128, s0 : s0 + SB], 0.0
                )
                nc.vector.tensor_scalar_mul(
                    kz2[0:64, s0 : s0 + SB], kvt[0:64, s0 : s0 + SB], 0.0
                )
                nc.sync.dma_start(
                    out=kz2[64:128, s0 : s0 + SB], in_=kvt[0:64, s0 : s0 + SB]
                )
                # V^T -> V [t, d] (+ ones col) for this s-block's t-tiles
                for tt in range(4 * sb, 4 * (sb + 1)):
                    pst = pp.tile([128, 64], F32, tag="work", name=f"pst_{b}_{tt}")
                    nc.tensor.transpose(
                        pst,
                        kvt[64:128, tt * 128 : (tt + 1) * 128].bitcast(F32),
                        ident[64:128, 64:128],
                    )
                    va = perb.tile(
                        [128, D + 1], F32R, tag="vaug", bufs=ntt_b, name=f"vaug_{b}_{tt}"
                    )
                    nc.vector.tensor_copy(va[:, 0:D], pst)
                    nc.vector.tensor_copy(va[:, D : D + 1], onec)
                    vaug.append(va)

            # ---- attention + out-projection, per sq-block ----
            for sb in range(nsb):
                s0 = sb * SB
                ntt = 4 * (sb + 1)
                pots = [
                    pp.tile([D + 1, SB], F32, tag="ot", bufs=4, name=f"pot_{b}_{sb}_{h}")
                    for h in range(HPC)
                ]
                ptss = {}

                def emit_scores(tt):
                    t0 = tt * 128
                    diag = tt >= 4 * sb
                    psses = []
                    for h in range(HPC):
                        hp, hh = divmod(h, 2)
                        pss = pp.tile(
                            [128, SB], F32, tag="work", name=f"pss_{b}_{sb}_{tt}_{h}"
                        )
                        kmat = kz if hh == 0 else kz2
                        lhsT = kmat[:, t0 : t0 + 128]
                        rhs = qc[hp][:, s0 : s0 + SB]
                        nc.tensor.matmul(pss, lhsT, rhs, start=True, stop=True)
                        psses.append(pss)
                    for h in range(HPC):
                        pt = stream.tile(
                            [128, SB], F32R, tag="p", bufs=8, name=f"p_{b}_{sb}_{tt}_{h}"
                        )
                        nc.scalar.activation(
                            pt, psses[h], EXP, scale=1.0 / np.sqrt(D)
                        )
                        if diag:
                            off = 384 - (t0 - s0)
                            nc.vector.tensor_mul(pt, pt, msk_s[:, off : off + SB])
                        ptss[(tt, h)] = pt

                def emit_pv(tt):
                    for h in range(HPC):
                        nc.tensor.matmul(
                            pots[h],
                            vaug[tt],
                            ptss.pop((tt, h)),
                            start=(tt == 0),
                            stop=(tt == ntt - 1),
                        )

                emit_scores(0)
                for tt in range(1, ntt):
                    emit_scores(tt)
                    drain(1)
                    emit_pv(tt - 1)
                    drain(1)
                emit_pv(ntt - 1)
                drain(len(pending))
                # normalize: batched reciprocal of 4 denom rows at
                # partitions 0/32/64/96, per-head PE ones-matmul broadcast
                otss = []
                den4 = stream.tile(
                    [97, SB], F32, tag="den4", bufs=2, name=f"den4_{b}_{sb}"
                )
                nc.vector.memset(den4, 1.0)
                for h in range(HPC):
                    ots = stream.tile(
                        [D + 1, SB], F32, tag="ots", bufs=6, name=f"ots_{b}_{sb}_{h}"
                    )
                    nc.vector.tensor_copy(ots, pots[h])
                    otss.append(ots)
                    nc.sync.dma_start(
                        out=den4[32 * h : 32 * h + 1, :], in_=ots[D : D + 1, :]
                    )
                rec4 = stream.tile(
                    [97, SB], F32R, tag="rec4", bufs=2, name=f"rec4_{b}_{sb}"
                )
                with nc.allow_low_precision(
                    reason="fp32r rounding of softmax reciprocal (~1e-4)"
                ):
                    nc.vector.reciprocal(rec4, den4)
                oscs = []
                for h in range(HPC):
                    pbc = pp.tile(
                        [64, SB], F32, tag="py", bufs=1, name=f"pbc_{b}_{sb}_{h}"
                    )
                    nc.tensor.matmul(
                        pbc,
                        ones97[32 * h : 32 * h + 1, :],
                        rec4[32 * h : 32 * h + 1, :],
                        start=True,
                        stop=True,
                        tile_position=(32 * h, 0),
                    )
                    osc = stream.tile(
                        [128, SB], F32R, tag="osc", bufs=6, name=f"osc_{b}_{sb}_{h}"
                    )
                    nc.vector.tensor_mul(osc[0:64, :], otss[h][0:D, :], pbc)
                    nc.vector.tensor_scalar_mul(
                        osc[64:128, :], qc[0][64:128, s0 : s0 + SB], 0.0
                    )
                    oscs.append(osc)
                # out projection: deferred chunks, drained inside the
                # next sq-block's attention loop to keep the PE dense
                def make_chunks(sb, oscs, bs0=bs0, b=b):
                    s0 = sb * SB
                    state = {}

                    def chunk(sc, dmc):
                        def run():
                            if dmc == 0:
                                state[sc] = stream.tile(
                                    [128, DM], F32, tag="ysb", bufs=2,
                                    name=f"ysb_{b}_{sb}_{sc}",
                                )
                            ysb = state[sc]
                            py = pp.tile(
                                [128, SB], F32, tag="py", bufs=1,
                                name=f"py_{b}_{sb}_{sc}_{dmc}",
                            )
                            for h in range(HPC):
                                nc.tensor.matmul(
                                    py,
                                    oscs[h][:, sc * 128 : (sc + 1) * 128],
                                    wo4_s[:, h, dmc * SB : (dmc + 1) * SB],
                                    start=(h == 0),
                                    stop=(h == HPC - 1),
                                )
                            nc.vector.tensor_copy(
                                ysb[:, dmc * SB : (dmc + 1) * SB], py
                            )
                            if dmc == DM // SB - 1:
                                srow = bs0 + s0 + sc * 128
                                nc.sync.dma_start(
                                    out=y[srow : srow + 128, :], in_=ysb
                                )
                        return run

                    return [
                        chunk(sc, dmc)
                        for sc in range(SB // 128)
                        for dmc in range(DM // SB)
                    ]

                pending.extend(make_chunks(sb, oscs))
        drain(len(pending))


def make_core_inputs(x, Wq, bq, Wk, bk, Wv, bv, Wo, b_count=B, s_len=S):
    """Host-side sharding: per-core input dicts (list of 8)."""
    f = np.float32
    x = np.asarray(x, f).reshape(b_count * s_len, DM)
    xt = np.ascontiguousarray(x.T)  # [DM, B*S]
    jj = np.arange(896)
    t = np.arange(128)
    msk = (t[:, None] <= (jj[None, :] - 384)).astype(f)
    maps = []
    for d in range(NCORES):
        Wqd = np.asarray(Wq, f)[:, d * 256 : (d + 1) * 256]
        wq_arr = np.ascontiguousarray(
            Wqd.reshape(KT, 128, 256).transpose(1, 0, 2)
        )  # [128, KT, 256]
        Wkd = np.asarray(Wk, f)[:, d * 64 : (d + 1) * 64]
        Wvd = np.asarray(Wv, f)[:, d * 64 : (d + 1) * 64]
        Wkvd = np.concatenate([Wkd, Wvd], axis=1)  # [DM, 128]
        wkv_arr = np.ascontiguousarray(Wkvd.reshape(KT, 128, 128).transpose(1, 0, 2))
        Wod = np.asarray(Wo, f)[d * 256 : (d + 1) * 256, :]
        wo4 = np.zeros((128, HPC, DM), f)
        wo4[0:64] = Wod.reshape(HPC, 64, DM).transpose(1, 0, 2)
        bq2 = np.ascontiguousarray(
            np.asarray(bq, f)[d * 256 : (d + 1) * 256].reshape(2, 128).T
        )
        bkv2 = np.ascontiguousarray(
            np.concatenate(
                [np.asarray(bk, f)[d * 64 : (d + 1) * 64], np.asarray(bv, f)[d * 64 : (d + 1) * 64]]
            ).reshape(128, 1)
        )
        maps.append(
            dict(xt=xt, wq=wq_arr, wkv=wkv_arr, wo4=wo4, bq2=bq2, bkv=bkv2, msk=msk)
        )
    return maps


_NC_CACHE = {}


def _get_nc(b_count=B, s_len=S):
    key = (b_count, s_len)
    if key not in _NC_CACHE:
        _NC_CACHE[key] = build_program(b_count, s_len)
    return _NC_CACHE[key]


def kernel(x, Wq, bq, Wk, bk, Wv, bv, Wo, bo):
    nc = _get_nc()
    in_maps = make_core_inputs(x, Wq, bq, Wk, bk, Wv, bv, Wo)
    res = run_bass_kernel_spmd(nc, in_maps, core_ids=list(range(NCORES)))
    acc = np.zeros((B * S, DM), np.float64)
    for r in res.results:
        acc += r["y"]
    acc += np.asarray(bo, np.float64)
    return acc.reshape(B, S, DM).astype(np.float32)
